# revision 1
# baseline (speedup 1.0000x reference)
"""Trainium2 Bass kernel for DeepSet segment-reduce — v2 (pipelined, ACT/DVE balanced).

Same host-side layout as the baseline (uniform-length segment classes, padded
by duplicating elements), but a rewritten device program:

- Two-phase software pipeline (phase2 runs 2 groups behind phase1) with
  emission interleaved so PE/ACT/DVE always have independent ready work.
- LeakyReLU evacuations split between ScalarE (Prelu) and VectorE
  (tensor_scalar bias-add + scalar_tensor_tensor fused leaky), balanced
  per class via an output-column split.
- Segment max via a binary tree of 2x-mode tensor_tensor max ops, falling
  back to tensor_reduce for odd factors.
- mlp3 layer-1 consumes a2 = 0.2*t2 + 0.8*relu(t2) via two prescaled
  matmul terms (t2, r2) instead of materializing a2.
- fp16 output (halves the out DMA); packed (gap-free) SBUF tiles.
"""
import numpy as np

import concourse.bass as bass
import concourse.mybir as mybir
import concourse.tile as tile
from concourse import bacc
from concourse.bass_utils import run_bass_kernel_spmd

N_CORES = 8
D_IN = 64
D_OUT = 128
ALPHA = 0.2
FD = 480                      # real columns per chunk (half-group)
GAP = 512                     # chunk stride inside a PSUM tile
SPAN = 2 * GAP                # PSUM tile free size (2 banks)
GROUP_COLS = 2 * FD           # real columns per group
CLASSES = [1, 2, 3, 4, 5, 6, 8, 10, 12, 15, 16, 20, 24, 30, 32, 40, 48]
LMAXC = 48

F16 = mybir.dt.float16
F32 = mybir.dt.float32
PR = mybir.ActivationFunctionType.Prelu
CPY = mybir.ActivationFunctionType.Copy
MAX = mybir.AluOpType.max
ADD = mybir.AluOpType.add
MULT = mybir.AluOpType.mult
AXX = mybir.AxisListType.X


# ----------------------------------------------------------------------------
# Host-side layout (identical to baseline)
# ----------------------------------------------------------------------------

def _next_class(lengths):
    cls = np.empty(len(lengths), dtype=np.int64)
    cls.fill(-1)
    for c in reversed(CLASSES):
        cls[lengths <= c] = c
    return cls


def build_layout(csr_idx):
    csr = np.asarray(csr_idx).astype(np.int64)
    counts = csr[1:] - csr[:-1]
    nz = counts > 0
    seg_start = csr[:-1][nz]
    seg_len = counts[nz]
    lmax = int(seg_len.max()) if len(seg_len) else 1

    slots_per_chunk_48 = FD // LMAXC  # 10
    if lmax > LMAXC:
        for p in (2, 5, 10):
            if p * LMAXC >= lmax and slots_per_chunk_48 % p == 0:
                p_max = p
                break
        else:
            raise ValueError(f"segment too long: {lmax}")
    else:
        p_max = 1

    is_split = seg_len > LMAXC
    norm_start, norm_len = seg_start[~is_split], seg_len[~is_split]
    sp_start, sp_len = seg_start[is_split], seg_len[is_split]

    n_split_core = int(np.ceil(len(sp_start) / N_CORES)) if p_max > 1 else 0
    if p_max > 1:
        per_half = slots_per_chunk_48 // p_max
        n_split_core = int(np.ceil(n_split_core / per_half)) * per_half

    cls = _next_class(norm_len)
    order = np.argsort(cls, kind="stable")
    cls_sorted = cls[order]
    start_sorted = norm_start[order]
    len_sorted = norm_len[order]

    core_slots = {c: [[] for _ in range(N_CORES)] for c in CLASSES}
    for c in CLASSES:
        m = cls_sorted == c
        st, ln = start_sorted[m], len_sorted[m]
        n = len(st)
        n_core = int(np.ceil(n / N_CORES)) if n else 0
        for core in range(N_CORES):
            s = st[core::N_CORES]
            l = ln[core::N_CORES]
            pad = n_core - len(s)
            if pad > 0:
                s = np.concatenate([s, np.zeros(pad, np.int64)])
                l = np.concatenate([l, np.zeros(pad, np.int64)])
            core_slots[c][core] = (s, l)

    groups = []
    elem_idx = [[] for _ in range(N_CORES)]
    orig_of = [[] for _ in range(N_CORES)]

    def expand(core, starts, lens, L):
        starts = np.asarray(starts, np.int64)
        lens = np.asarray(lens, np.int64)
        j = np.arange(L)[None, :]
        last = np.maximum(lens - 1, 0)[:, None]
        idx = starts[:, None] + np.minimum(j, last)
        org = np.where(j < lens[:, None], starts[:, None] + j, -1)
        elem_idx[core].append(idx.ravel())
        orig_of[core].append(org.ravel())

    for c in CLASSES:
        per_core_slots = []
        for core in range(N_CORES):
            s, l = core_slots[c][core]
            if c == LMAXC and p_max > 1:
                ss = sp_start[core::N_CORES]
                sl = sp_len[core::N_CORES]
                pad = n_split_core - len(ss)
                if pad > 0:
                    ss = np.concatenate([ss, np.zeros(pad, np.int64)])
                    sl = np.concatenate([sl, np.zeros(pad, np.int64)])
                pst, pln = [], []
                for k in range(p_max):
                    off = k * LMAXC
                    pl = np.clip(sl - off, 0, LMAXC)
                    ps = np.where(pl > 0, ss + off, ss)
                    pst.append(ps)
                    pln.append(pl)
                pst = np.stack(pst, 1).ravel()
                pln = np.stack(pln, 1).ravel()
                s = np.concatenate([pst, s])
                l = np.concatenate([pln, l])
            per_core_slots.append((s, l))

        n_slots = max(len(s) for s, _ in per_core_slots)
        seg_per_group = GROUP_COLS // c
        n_slots = int(np.ceil(n_slots / seg_per_group)) * seg_per_group if n_slots else 0
        if n_slots == 0:
            continue
        for core in range(N_CORES):
            s, l = per_core_slots[core]
            pad = n_slots - len(s)
            if pad > 0:
                s = np.concatenate([s, np.zeros(pad, np.int64)])
                l = np.concatenate([l, np.zeros(pad, np.int64)])
            expand(core, s, l, c)
        n_groups = n_slots // seg_per_group
        total_fix_slots = n_split_core * p_max if (c == LMAXC and p_max > 1) else 0
        spc = FD // c
        for g in range(n_groups):
            nfix = [0, 0]
            for h in range(2):
                lo = (g * 2 + h) * spc
                hi = lo + spc
                nf = min(max(total_fix_slots - lo, 0), spc)
                nfix[h] = nf // p_max
            groups.append((c, nfix[0], nfix[1]))

    for core in range(N_CORES):
        elem_idx[core] = np.concatenate(elem_idx[core])
        orig_of[core] = np.concatenate(orig_of[core])
    elem_idx = np.stack(elem_idx)
    orig_of = np.stack(orig_of)

    E = elem_idx.shape[1]
    assert E % GROUP_COLS == 0 and E // GROUP_COLS == len(groups)
    return dict(elem_idx=elem_idx, orig_of=orig_of, groups=groups,
                p_max=p_max, E=E)


# ----------------------------------------------------------------------------
# Device program
# ----------------------------------------------------------------------------

WNAMES = ["w11", "w12", "w21", "w21P", "w21R", "w22",
          "w31aP", "w31aR", "w31b", "w32"]
WDIMS = [D_IN + 1] + [D_OUT] * 9
BNAMES = ["b12", "b21", "b22", "b31", "b32"]

import os as _os
ACT_PAIR = float(_os.environ.get("K2_ACT_PAIR", "780"))
DVE_COPY = float(_os.environ.get("K2_DVE_COPY", "950"))
DVE_LEAKY = float(_os.environ.get("K2_DVE_LEAKY", "1920"))


def build_nc(groups, p_max, E, loop_n=1):
    nc = bacc.Bacc("TRN2", target_bir_lowering=False, debug=False)

    xin = nc.declare_dram_parameter("xin", [D_IN + 1, E], F16, isOutput=False)
    out = nc.declare_dram_parameter("out", [D_OUT, E], F16, isOutput=True)
    wp = {n: nc.declare_dram_parameter(n, [k, D_OUT], F16, isOutput=False)
          for n, k in zip(WNAMES, WDIMS)}
    bp = {n: nc.declare_dram_parameter(n, [D_OUT, 1], F32, isOutput=False)
          for n in BNAMES}

    with tile.TileContext(nc) as tc:
        with (
            tc.tile_pool(name="wpool", bufs=1) as wpool,
            tc.tile_pool(name="xpool", bufs=6) as xpool,
            tc.tile_pool(name="spool", bufs=6) as spool,
            tc.tile_pool(name="opool", bufs=4) as opool,
            tc.tile_pool(name="pbig", bufs=2, space="PSUM") as pbig,
        ):
            wt = {}
            for n, k in zip(WNAMES, WDIMS):
                wt[n] = wpool.tile([k, D_OUT], F16, tag=f"w_{n}", name=f"w_{n}")
                nc.gpsimd.dma_start(wt[n][:], wp[n][:])
            bt = {}
            for n in BNAMES:
                bt[n] = wpool.tile([D_OUT, 1], F32, tag=f"b_{n}", name=f"b_{n}")
                nc.gpsimd.dma_start(bt[n][:], bp[n][:])
            ones = wpool.tile([1, FD], F16, tag="ones", name="ones")
            nc.vector.memset(ones[:], 1.0)

            import contextlib
            loop_ctx = (tc.For_i(0, loop_n, 1) if loop_n > 1
                        else contextlib.nullcontext())
            with loop_ctx:
                body(nc, tc, groups, p_max, wt, bt, xin, out,
                     xpool, spool, opool, pbig, ones)

    nc.finalize()
    return nc


def body(nc, tc, groups, p_max, wt, bt, xin, out,
         xpool, spool, opool, pbig, ones):
    n = len(groups)
    st = {}
    load = {"act": 0.0, "dve": 0.0}

    def pick(cost_a, cost_d):
        """Greedy engine choice for one half-evacuation."""
        if max(load["act"] + cost_a, load["dve"]) <= max(load["act"],
                                                         load["dve"] + cost_d):
            load["act"] += cost_a
            return "act"
        load["dve"] += cost_d
        return "dve"

    # --- pieces --------------------------------------------------------------
    def hv(t):
        """[p, SPAN] psum tile -> [p, 2, FD] strided view (skips gaps)."""
        return t[:].rearrange("p (h c) -> p h c", h=2, c=GAP)[:, :, :FD]

    def pk(t):
        """[p, >=GROUP_COLS] sbuf tile -> [p, 2, FD] packed view."""
        return t[:, :GROUP_COLS].rearrange("p (h c) -> p h c", h=2, c=FD)

    def dma_in(g):
        xt = xpool.tile([D_IN + 1, GROUP_COLS], F16, tag="xt", name="xt")
        nc.sync.dma_start(xt[:], xin[:, g * GROUP_COLS:(g + 1) * GROUP_COLS])
        st[g] = dict(xt=xt)

    def pe_u1(g):
        u1 = pbig.tile([D_OUT, SPAN], F32, tag="u12", name="u12", bufs=2)
        xt = st[g]["xt"]
        for h in range(2):
            nc.tensor.matmul(u1[:, h * GAP:h * GAP + FD], wt["w11"][:],
                             xt[:, h * FD:(h + 1) * FD], start=True, stop=True)
        st[g]["u1"] = u1

    def evac_a1(g):
        # u1 is pre-biased (ones row in xin); leaky-evacuate both halves.
        a1 = spool.tile([D_OUT, GROUP_COLS], F16, tag="a1", name="a1")
        st[g]["a1"] = a1
        uv, ov = hv(st[g]["u1"]), pk(a1)
        if pick(ACT_PAIR, DVE_LEAKY) == "act":
            nc.scalar.activation(ov, uv, PR, bias=0.0, scale=1.0, alpha=ALPHA)
        else:
            t1 = spool.tile([D_OUT, GROUP_COLS], F16, tag="tev", name="t1")
            nc.vector.tensor_copy(pk(t1), uv)
            nc.vector.scalar_tensor_tensor(ov, pk(t1), ALPHA, pk(t1),
                                           op0=MULT, op1=MAX)

    def pe_u2(g):
        u2 = st[g]["u1"]
        a1 = st[g]["a1"]
        for h in range(2):
            nc.tensor.matmul(u2[:, h * GAP:h * GAP + FD], wt["w12"][:],
                             a1[:, h * FD:(h + 1) * FD], start=True, stop=True)
        st[g]["u2"] = u2

    def evac_t2(g):
        # t2 = u2 + b12 (pre-activation), fp16, feeds pool/r2/u5.
        t2 = spool.tile([D_OUT, GROUP_COLS], F16, tag="t2", name="t2")
        st[g]["t2"] = t2
        uv, ov = hv(st[g]["u2"]), pk(t2)
        if pick(ACT_PAIR, DVE_COPY) == "act":
            nc.scalar.activation(ov, uv, PR, bias=bt["b12"][:], scale=1.0,
                                 alpha=1.0)
        else:
            nc.vector.tensor_scalar_add(ov, uv, bt["b12"][:])

    def dve_r2(g):
        r2 = spool.tile([D_OUT, GROUP_COLS], F16, tag="r2", name="r2")
        nc.vector.tensor_scalar_max(r2[:], st[g]["t2"][:], 0.0)
        st[g]["r2"] = r2
        load["dve"] += 310

    def dve_pool(g, L):
        m = FD // L
        cur = st[g]["t2"][:, :GROUP_COLS].rearrange(
            "p (h q l) -> p h q l", h=2, q=m, l=L)
        Lc = L
        si = 0
        cost = 0
        while Lc % 2 == 0 and Lc > 2:
            Lc //= 2
            nxt_t = spool.tile([D_OUT, FD], F16, tag=f"ptree{si % 2}",
                               name="ptree")
            nxt = nxt_t[:, :2 * m * Lc].rearrange(
                "p (h q l) -> p h q l", h=2, q=m, l=Lc)
            nc.vector.tensor_tensor(nxt, cur[:, :, :, :Lc],
                                    cur[:, :, :, Lc:2 * Lc], op=MAX)
            cost += 60 + m * Lc
            cur = nxt
            si += 1
        pooled = spool.tile([D_OUT, FD], F16, tag="pooled_t", name="pooled_t")
        pview = pooled[:, :2 * m].rearrange("p (h q) -> p h q", h=2, q=m)
        if Lc == 2:
            nc.vector.tensor_tensor(pview, cur[:, :, :, 0], cur[:, :, :, 1],
                                    op=MAX)
            cost += 60 + 2 * m
        elif Lc == 3:
            tmp_t = spool.tile([D_OUT, FD], F16, tag=f"ptree{si % 2}",
                               name="ptree3")
            tmp = tmp_t[:, :2 * m].rearrange("p (h q) -> p h q", h=2, q=m)
            nc.vector.tensor_tensor(tmp, cur[:, :, :, 0], cur[:, :, :, 1],
                                    op=MAX)
            nc.vector.tensor_tensor(pview, tmp, cur[:, :, :, 2], op=MAX)
            cost += 2 * (60 + 2 * m)
        else:
            nc.vector.tensor_reduce(pview, cur, axis=AXX, op=MAX)
            cost += 60 + 2 * m * Lc
        st[g]["pooled_t"] = pooled
        load["dve"] += cost

    def dve_fix(g, L, nfix0, nfix1):
        m = FD // L
        pooled = st[g]["pooled_t"]
        for h, nfix in ((0, nfix0), (1, nfix1)):
            if nfix == 0:
                continue
            off = h * m
            tmp = spool.tile([D_OUT, FD // LMAXC], F16, tag="fixtmp",
                             name="fixtmp")
            nc.vector.tensor_reduce(
                tmp[:, :nfix],
                pooled[:, off:off + nfix * p_max].rearrange(
                    "p (k q) -> p k q", k=nfix, q=p_max),
                axis=AXX, op=MAX)
            nc.vector.tensor_copy(
                pooled[:, off:off + nfix * p_max].rearrange(
                    "p (k q) -> p k q", k=nfix, q=p_max),
                tmp[:, :nfix].unsqueeze(2).broadcast_to([D_OUT, nfix, p_max]))
            load["dve"] += 260

    def dve_pooled_a(g, L):
        m = FD // L
        pa = spool.tile([D_OUT, FD], F16, tag="pooled_a", name="pooled_a")
        nc.vector.scalar_tensor_tensor(
            pa[:, :2 * m], st[g]["pooled_t"][:, :2 * m], ALPHA,
            st[g]["pooled_t"][:, :2 * m], op0=MULT, op1=MAX)
        st[g]["pooled_a"] = pa
        load["dve"] += 60 + m

    def pe_u3(g, L):
        if L == 1:
            u3 = pbig.tile([D_OUT, SPAN], F32, tag="u12", name="u34L1", bufs=2)
            for h in range(2):
                sl = slice(h * FD, (h + 1) * FD)
                dst = u3[:, h * GAP:h * GAP + FD]
                nc.tensor.matmul(dst, wt["w21P"][:], st[g]["t2"][:, sl],
                                 start=True, stop=False)
                nc.tensor.matmul(dst, wt["w21R"][:], st[g]["r2"][:, sl],
                                 start=False, stop=True)
        else:
            m = FD // L
            u3 = pbig.tile([D_OUT, GAP], F32, tag="u12", name="uset", bufs=2)
            pa = st[g]["pooled_a"]
            load["act"] += (2 * m + 112) / 1.2
            nc.tensor.matmul(u3[:, :2 * m], wt["w21"][:], pa[:, :2 * m],
                             start=True, stop=True)
        st[g]["u3"] = u3

    def act_a3(g, L):
        a3 = spool.tile([D_OUT, GROUP_COLS], F16, tag="a3", name="a3")
        if L == 1:
            nc.scalar.activation(
                a3[:, :GROUP_COLS].rearrange("p (h c) -> p h c", h=2, c=FD),
                st[g]["u3"][:].rearrange("p (h c) -> p h c", h=2, c=GAP)[:, :, :FD],
                PR, bias=bt["b21"][:], scale=1.0, alpha=ALPHA)
            load["act"] += ACT_PAIR
        else:
            m = FD // L
            nc.scalar.activation(a3[:, :2 * m], st[g]["u3"][:, :2 * m],
                                 PR, bias=bt["b21"][:], scale=1.0,
                                 alpha=ALPHA)
        st[g]["a3"] = a3

    def pe_u4(g, L):
        a3 = st[g]["a3"]
        u4 = st[g]["u3"]
        if L == 1:
            for h in range(2):
                nc.tensor.matmul(u4[:, h * GAP:h * GAP + FD], wt["w22"][:],
                                 a3[:, h * FD:(h + 1) * FD],
                                 start=True, stop=True)
        else:
            m = FD // L
            load["act"] += (2 * m + 112) / 1.2
            nc.tensor.matmul(u4[:, :2 * m], wt["w22"][:], a3[:, :2 * m],
                             start=True, stop=True)
        st[g]["u4"] = u4

    def act_a4(g, L):
        a4 = spool.tile([D_OUT, GROUP_COLS], F16, tag="a4", name="a4")
        if L == 1:
            nc.scalar.activation(
                a4[:, :GROUP_COLS].rearrange("p (h c) -> p h c", h=2, c=FD),
                st[g]["u4"][:].rearrange("p (h c) -> p h c", h=2, c=GAP)[:, :, :FD],
                PR, bias=bt["b22"][:], scale=1.0, alpha=ALPHA)
            load["act"] += ACT_PAIR
        else:
            m = FD // L
            nc.scalar.activation(a4[:, :2 * m], st[g]["u4"][:, :2 * m],
                                 PR, bias=bt["b22"][:], scale=1.0,
                                 alpha=ALPHA)
        st[g]["a4"] = a4

    def pe_u5(g, L):
        u5 = pbig.tile([D_OUT, SPAN], F32, tag="u56", name="u56", bufs=2)
        m = FD // L
        t2, r2, a4 = st[g]["t2"], st[g]["r2"], st[g]["a4"]
        eng = pick(ACT_PAIR, DVE_LEAKY)
        st[g]["a5eng"] = eng
        for h in range(2):
            sl = slice(h * FD, (h + 1) * FD)
            dst = u5[:, h * GAP:h * GAP + FD]
            nc.tensor.matmul(dst, wt["w31aP"][:], t2[:, sl],
                             start=True, stop=False)
            nc.tensor.matmul(dst, wt["w31aR"][:], r2[:, sl],
                             start=False, stop=False)
            if L == 1:
                rhs = a4[:, sl]
            else:
                rhs = a4[:, h * m:(h + 1) * m].unsqueeze(2).broadcast_to(
                    [D_OUT, m, L])
            nc.tensor.matmul(dst, wt["w31b"][:], rhs, start=False, stop=True)
        st[g]["u5"] = u5

    def evac_a5(g):
        a5 = spool.tile([D_OUT, GROUP_COLS], F16, tag="a5", name="a5")
        st[g]["a5"] = a5
        uv, ov = hv(st[g]["u5"]), pk(a5)
        if st[g]["a5eng"] == "act":
            nc.scalar.activation(ov, uv, PR, bias=bt["b31"][:], scale=1.0,
                                 alpha=ALPHA)
        else:
            t5 = spool.tile([D_OUT, GROUP_COLS], F16, tag="tev", name="t5")
            nc.vector.tensor_scalar_add(pk(t5), uv, bt["b31"][:])
            nc.vector.scalar_tensor_tensor(ov, pk(t5), ALPHA, pk(t5),
                                           op0=MULT, op1=MAX)

    def pe_u6(g):
        u6 = st[g]["u5"]
        a5 = st[g]["a5"]
        eng = pick(ACT_PAIR, DVE_LEAKY)
        st[g]["outeng"] = eng
        for h in range(2):
            dst = u6[:, h * GAP:h * GAP + FD]
            nc.tensor.matmul(dst, wt["w32"][:], a5[:, h * FD:(h + 1) * FD],
                             start=True, stop=True)
        st[g]["u6"] = u6

    def evac_out(g):
        uv, ov = hv(st[g]["u6"]), pk(st[g]["ot"])
        if st[g]["outeng"] == "act":
            nc.scalar.activation(ov, uv, PR, bias=bt["b32"][:], scale=1.0,
                                 alpha=ALPHA)
        else:
            t6 = spool.tile([D_OUT, GROUP_COLS], F16, tag="tev", name="t6")
            nc.vector.tensor_scalar_add(pk(t6), uv, bt["b32"][:])
            nc.vector.scalar_tensor_tensor(ov, pk(t6), ALPHA, pk(t6),
                                           op0=MULT, op1=MAX)

    def dma_out(g):
        nc.sync.dma_start(out[:, g * GROUP_COLS:(g + 1) * GROUP_COLS],
                          st[g]["ot"][:, :GROUP_COLS])
        del st[g]

    # --- interleaved emission ------------------------------------------------
    import os
    L1B = int(os.environ.get("K2_L1B", "1"))   # u2/t2/pool/r2 stage lag
    L2 = int(os.environ.get("K2_L2", "2"))     # u3/a3/u4/a4 stage lag
    SPLIT = int(os.environ.get("K2_SPLIT", "2"))  # stagger u5, u6 stages
    L5 = L2 + SPLIT                            # u5/a5 lag
    L6 = L5 + SPLIT                            # u6 lag
    LO = L6 + 1                                # out evac + dma lag

    def grp(s, lag):
        i = s - lag
        return i if 0 <= i < n else None

    for s in range(n + LO):
        g0 = grp(s, 0)
        g1 = grp(s, L1B)
        g2 = grp(s, L2)
        g5 = grp(s, L5)
        g6 = grp(s, L6)
        go = grp(s, LO)

        if g0 is not None:
            dma_in(g0)
        if go is not None:
            evac_out(go)
        if g0 is not None:
            pe_u1(g0)
            evac_a1(g0)
        if g2 is not None:
            L_2 = groups[g2][0]
            pe_u3(g2, L_2)
            act_a3(g2, L_2)
        if g1 is not None:
            pe_u2(g1)
            evac_t2(g1)
        if g2 is not None:
            pe_u4(g2, L_2)
            act_a4(g2, L_2)
        if g1 is not None:
            L_1, nf0, nf1 = groups[g1]
            if L_1 > 1:
                dve_pool(g1, L_1)
        if g5 is not None:
            pe_u5(g5, groups[g5][0])
        if g1 is not None:
            dve_r2(g1)
        if g5 is not None:
            evac_a5(g5)
        if g1 is not None:
            if L_1 > 1:
                dve_fix(g1, L_1, nf0, nf1)
                dve_pooled_a(g1, L_1)
        if g6 is not None:
            pe_u6(g6)
            st[g6]["ot"] = opool.tile([D_OUT, GROUP_COLS], F16, tag="ot",
                                      name="ot")
        if go is not None:
            dma_out(go)


# ----------------------------------------------------------------------------
# Entry point
# ----------------------------------------------------------------------------

_CACHE = {}


def prepare(x, csr_idx, w11, s11, b11, w12, s12, b12,
            w21, s21, b21, w22, s22, b22,
            w31, s31, b31, w32, s32, b32, loop_n=1):
    x = np.asarray(x)
    lay = build_layout(csr_idx)
    E = lay["E"]

    key = (tuple(lay["groups"]), lay["p_max"], E, loop_n)
    if key not in _CACHE:
        _CACHE[key] = build_nc(lay["groups"], lay["p_max"], E, loop_n=loop_n)
    nc = _CACHE[key]

    def wprep(w, s):
        return (np.asarray(w) * np.asarray(s)[None, :]).astype(np.float16)

    w11f = wprep(w11, s11)
    w12f = wprep(w12, s12)
    w21f = wprep(w21, s21)
    w22f = wprep(w22, s22)
    w31f = wprep(w31, s31)
    w32f = wprep(w32, s32)
    w31a = np.ascontiguousarray(w31f[:D_OUT]).astype(np.float32)
    w11a = np.concatenate([w11f, np.asarray(b11, np.float16).reshape(1, D_OUT)])
    params = {
        "w11": w11a, "w12": w12f, "w21": w21f,
        "w21P": (ALPHA * w21f.astype(np.float32)).astype(np.float16),
        "w21R": ((1 - ALPHA) * w21f.astype(np.float32)).astype(np.float16),
        "w22": w22f,
        "w31aP": (ALPHA * w31a).astype(np.float16),
        "w31aR": ((1 - ALPHA) * w31a).astype(np.float16),
        "w31b": np.ascontiguousarray(w31f[D_OUT:]),
        "w32": w32f,
        "b12": np.asarray(b12, np.float32).reshape(D_OUT, 1),
        "b21": np.asarray(b21, np.float32).reshape(D_OUT, 1),
        "b22": np.asarray(b22, np.float32).reshape(D_OUT, 1),
        "b31": np.asarray(b31, np.float32).reshape(D_OUT, 1),
        "b32": np.asarray(b32, np.float32).reshape(D_OUT, 1),
    }

    x16 = x.astype(np.float16)
    in_maps = []
    for core in range(N_CORES):
        E1 = lay["elem_idx"].shape[1]
        xc = np.empty((D_IN + 1, E1), np.float16)
        xc[:D_IN] = x16[lay["elem_idx"][core]].T
        xc[D_IN] = 1.0
        in_maps.append({"xin": xc, **params})
    return nc, in_maps, lay


def run_device(nc, in_maps):
    return run_bass_kernel_spmd(nc, in_maps, list(range(N_CORES)))


def postprocess(res, lay, n):
    outp = np.empty((n, D_OUT), np.float32)
    filled = np.zeros(n, bool)
    for core in range(N_CORES):
        o = lay["orig_of"][core]
        m = o >= 0
        outp[o[m]] = res.results[core]["out"][:, m].T
        filled[o[m]] = True
    assert filled.all(), f"missing {int((~filled).sum())} elements"
    return outp


def kernel(x, csr_idx, **kw):
    x = np.asarray(x)
    nc, in_maps, lay = prepare(x, csr_idx, **kw)
    res = run_device(nc, in_maps)
    return postprocess(res, lay, x.shape[0])

